# revision 9
# baseline (speedup 1.0000x reference)
"""CSWM transition GNN kernel for 8 TRN2 NeuronCores.

Sharding: data-parallel over the 512 edge-groups (the quirky edge list is
block-diagonal over groups of 15 consecutive flat rows). Each core gets
64 groups + 64 of the 512 zero-agg tail rows = 1024 node rows, arranged
in a padded layout: 8 blocks x (120 edge rows + 8 tail rows) so every
block is 128-aligned. No cross-core communication.

Host-side algebra:
  - cat(xi,xi,xj)@e_w0 = xi@(W0a+W0b) + xj@W0c          (per-node U,V)
  - final edge matmul commutes with scatter-add; W2 then folds into the
    node MLP first layer: nw0s = e_w2 @ n_w0[532:1556]
  - LayerNorm mean is free: row-center W1 over its output dim (W1c) so
    the pre-LN activations are exactly zero-mean; only variance is needed
  - fp8 subnormal dodge: fp8 weights are pre-scaled (x16 / x8) host-side
    and the scale is undone in the eviction (LN is scale-invariant, so
    LN'd paths need only a scaled eps). Node weights are x16 in bf16
    (exact shift) so one eps tile serves both LNs.
  - edge slots are packed 224/group (the (14,14) diagonal slot is
    dropped) so each 8-group block is exactly 14 x 128-edge chunks
  - sT / z2T transposes ride the DMA XBAR (128x128 bf16), not the PE
  - per-edge work: one 1024x1024 fp8 DoubleRow matmul + LN + relu
"""

import numpy as np
import ml_dtypes

import concourse.bass as bass
import concourse.mybir as mybir
import concourse.tile as tile
from concourse import bacc
from concourse.bass_utils import run_bass_kernel_spmd
from concourse.masks import make_identity

BF16 = mybir.dt.bfloat16
F32 = mybir.dt.float32
F8 = mybir.dt.float8e4
DR = mybir.MatmulPerfMode.DoubleRow
AF = mybir.ActivationFunctionType
ADD = mybir.AluOpType.add

P = 128
D = 512            # embedding dim
H = 1024           # hidden dim
A_DIM = 20         # action dim
B = 512            # batch
K = 16             # objects
NG = 512           # total edge groups (block-diag over 15-row groups)
N_CORES = 8
G_CORE = NG // N_CORES          # 64 groups per core
EDGE_ROWS = G_CORE * 15         # 960 real edge rows per core
N_ROWS = 1024                   # padded: 8 blocks x (120 edge + 8 tail)
GB = 8                          # groups per aggregation block
NBLK = G_CORE // GB             # 8 blocks per core
SLOTS = 224                     # edge slots per group ((14,14) diag dropped)
E_BLK = GB * SLOTS              # 1792 = 14 x 128 exactly
NCHUNK = E_BLK // P             # 14 chunks of 128 edge-slots
EPS = 1e-5
W_SCALE = 16.0                  # host prescale on centered e_w1 and node weights
UV_SCALE = 8.0                  # host prescale on wab/w0c


def _bf16(x):
    return np.ascontiguousarray(np.asarray(x, dtype=np.float32).astype(ml_dtypes.bfloat16))


def _f8(x):
    return np.ascontiguousarray(np.asarray(x, dtype=np.float32).astype(ml_dtypes.float8_e4m3))


def _f32(x):
    return np.ascontiguousarray(np.asarray(x, dtype=np.float32))


def _build_amat():
    """[NCHUNK*128, 128] 0/1 matrix for the packed 224-slot layout:
    slot (gb, i<14, j) = gb*224 + i*15 + j -> node gb*15+i   (j != i)
    slot (gb, 14, j<14) = gb*224 + 210 + j -> node gb*15+14
    Node columns 120..127 stay zero (the per-block tail rows).
    """
    a = np.zeros((NCHUNK * P, P), dtype=np.float32)
    for gb in range(GB):
        base = gb * SLOTS
        for i in range(14):
            for j in range(15):
                if i != j:
                    a[base + i * 15 + j, gb * 15 + i] = 1.0
        for j in range(14):
            a[base + 210 + j, gb * 15 + 14] = 1.0
    return a


def _build_program(trivial_affine_e: bool, trivial_affine_n: bool):
    nc = bacc.Bacc("TRN2", target_bir_lowering=False, debug=False)

    # ---- DRAM parameters (per-core shards / replicated weights) ----
    # weight tensors are pre-transposed host-side to [p, k, n] so every
    # load is a plain contiguous HW-DGE DMA (no gpsimd gather).
    def din(name, shape, dt):
        return nc.declare_dram_parameter(name, list(shape), dt, isOutput=False)

    xT = din("xT", (P, 4, N_ROWS), F8)         # x transposed, [p,ks,rows]
    actT = din("actT", (A_DIM + 1, N_ROWS), BF16)   # one-hot actions + edge-row indicator
    wab = din("wab", (P, 4, H), F8)            # (W0a+W0b)*UV_SCALE
    w0c = din("w0c", (P, 4, H), F8)
    b0 = din("b0", (H,), F32)
    w1 = din("w1", (P, 8, H), F8)              # centered, *W_SCALE
    b1 = din("b1", (1, H), F8)                 # centered, *W_SCALE
    amat = din("amat", (P, NCHUNK, P), F8)
    nw0x = din("nw0x", (P, 4, H), F8)          # *W_SCALE
    nw0a = din("nw0a", (A_DIM + 1, H), BF16)   # rows 0..19 action, row 20 = e_b2 @ n_w0s; *W_SCALE
    nw0s = din("nw0s", (P, 8, H), BF16)        # *W_SCALE
    nb0 = din("nb0", (H,), F32)                # unscaled
    nw1 = din("nw1", (P, 8, H), BF16)          # centered, *W_SCALE
    nb1 = din("nb1", (1, H), BF16)             # centered, *W_SCALE
    nw2 = din("nw2", (P, 8, D), BF16)          # *W_SCALE
    nb2 = din("nb2", (1, D), BF16)             # *W_SCALE
    if not trivial_affine_e:
        e_g = din("e_g", (H,), F32)
        e_be = din("e_be", (H,), F32)
    if not trivial_affine_n:
        n_g = din("n_g", (H,), F32)
        n_be = din("n_be", (H,), F32)

    out = nc.declare_dram_parameter("out", [N_ROWS, D], BF16, isOutput=True)

    with tile.TileContext(nc) as tc:
        with (
            tc.tile_pool(name="const", bufs=1) as cpool,
            tc.tile_pool(name="nw", bufs=1) as nw,
        ):
            # ---- edge-critical loads first (sync queue) ----
            xT_s = cpool.tile([P, 4, N_ROWS], F8)
            nc.sync.dma_start(xT_s[:], xT[:])
            ones_row = cpool.tile([1, P], BF16)
            nc.vector.memset(ones_row[:], 1.0)
            eps_t = cpool.tile([P, 1], F32)
            nc.vector.memset(eps_t[:], EPS * W_SCALE * W_SCALE)
            # sT: aggregated-hidden, transposed [feat, rows]
            sT = cpool.tile([P, 8, N_ROWS], BF16)
            actT_s = cpool.tile([A_DIM + 1, N_ROWS], BF16)
            nc.sync.dma_start(actT_s[:], actT[:])

            # ---- node-phase weights on the scalar DMA queue (parallel) ----
            nw0x_s = nw.tile([P, 4, H], F8)
            nc.scalar.dma_start(nw0x_s[:], nw0x[:])
            nw0a_s = nw.tile([A_DIM + 1, H], BF16)
            nc.scalar.dma_start(nw0a_s[:], nw0a[:])
            nw0s_s = nw.tile([P, 8, H], BF16)
            nc.scalar.dma_start(nw0s_s[:], nw0s[:])
            nw1_s = nw.tile([P, 8, H], BF16)
            nc.scalar.dma_start(nw1_s[:], nw1[:])
            nw2_s = nw.tile([P, 8, D], BF16)
            nc.scalar.dma_start(nw2_s[:], nw2[:])
            nb0_t = nw.tile([P, 8], F32)
            nc.scalar.dma_start(nb0_t[:], nb0[:].rearrange("(o p) -> p o", p=P))
            nb1_r = nw.tile([1, H], BF16)
            nc.scalar.dma_start(nb1_r[:], nb1[:])
            nb2_s = nw.tile([1, D], BF16)
            nc.scalar.dma_start(nb2_s[:], nb2[:])
            if not trivial_affine_n:
                ng_b = nw.tile([P, H], F32)
                nc.scalar.dma_start(ng_b[:], n_g[None, :].to_broadcast((P, H)))
                nbe_b = nw.tile([P, H], F32)
                nc.scalar.dma_start(nbe_b[:], n_be[None, :].to_broadcast((P, H)))

            # ================= EDGE PHASE =================
            with (
                tc.tile_pool(name="ew", bufs=1) as ew,
                tc.tile_pool(name="uv", bufs=1) as uvp,
                tc.tile_pool(name="rp", bufs=2) as rp,
                tc.tile_pool(name="zp", bufs=5) as zp,
                tc.tile_pool(name="st", bufs=3) as stp,
                tc.tile_pool(name="ps", bufs=3, space="PSUM") as ps,
                tc.tile_pool(name="pa", bufs=1, space="PSUM") as pa,
            ):
                wab_s = ew.tile([P, 4, H], F8)
                nc.sync.dma_start(wab_s[:], wab[:])
                w0c_s = ew.tile([P, 4, H], F8)
                nc.sync.dma_start(w0c_s[:], w0c[:])
                b0_t = ew.tile([P, 8], F32)
                nc.sync.dma_start(b0_t[:], b0[:].rearrange("(o p) -> p o", p=P))
                w1_s = ew.tile([P, 8, H], F8)
                nc.sync.dma_start(w1_s[:], w1[:])
                amat_s = ew.tile([P, NCHUNK, P], F8)
                nc.sync.dma_start(amat_s[:], amat[:])
                b1_r = ew.tile([1, H], F8)
                nc.sync.dma_start(b1_r[:], b1[:])
                ones8 = ew.tile([1, P], F8)
                nc.vector.memset(ones8[:], 1.0)
                if not trivial_affine_e:
                    eg_b = ew.tile([P, H], F32)
                    nc.sync.dma_start(eg_b[:], e_g[None, :].to_broadcast((P, H)))
                    ebe_b = ew.tile([P, H], F32)
                    nc.sync.dma_start(ebe_b[:], e_be[None, :].to_broadcast((P, H)))

                # ---- U = x@(W0a+W0b)+b0, V = x@W0c  (transposed, fp8 DR) ----
                # half-major order so block-0..3 r-build can start early
                u_s = uvp.tile([P, 8, N_ROWS], BF16, tag="u")
                v_s = uvp.tile([P, 8, N_ROWS], BF16, tag="v")
                for half in (0, 512):
                    for m in range(8):
                        for dst, wt, bias in ((u_s, wab_s, True), (v_s, w0c_s, False)):
                            pt = ps.tile([P, H], F32, tag="mm")
                            for kp in range(2):
                                nc.tensor.matmul(
                                    pt[:, 0:512],
                                    wt[:, 2 * kp:2 * kp + 2, m * P:(m + 1) * P],
                                    xT_s[:, 2 * kp:2 * kp + 2, half:half + 512],
                                    start=(kp == 0), stop=(kp == 1), perf_mode=DR,
                                )
                            nc.scalar.activation(
                                dst[:, m, half:half + 512], pt[:, 0:512], AF.Identity,
                                bias=b0_t[:, m:m + 1] if bias else 0.0,
                                scale=1.0 / UV_SCALE,
                            )

                # ---- per-block: build r, edge matmul + LN, aggregate ----
                s_blks = []

                def emit_agg_pair(pagg, cp, zpair):
                    # chunks (2cp, 2cp+1) in one DoubleRow matmul, K=256
                    lhs = amat_s[:, 2 * cp:2 * cp + 2, :]
                    for half in (0, 512):
                        nc.tensor.matmul(pagg[:, half:half + 512], lhs,
                                         zpair[:, :, half:half + 512],
                                         start=(cp == 0), stop=(cp == NCHUNK // 2 - 1),
                                         perf_mode=DR)

                for blk in range(NBLK):
                    r_t = rp.tile([P, 8, E_BLK], F8, tag="r")
                    col0 = blk * P
                    for fs in range(8):
                        u_g = u_s[:, fs, col0:col0 + 120].rearrange("p (g i) -> p g i", i=15)
                        v_g = v_s[:, fs, col0:col0 + 120].rearrange("p (g j) -> p g j", j=15)
                        rb = stp.tile([P, E_BLK], BF16, tag="rb")
                        rb_g = rb[:].rearrange("p (g s) -> p g s", s=SLOTS)
                        out1 = rb_g[:, :, 0:210].rearrange("p g (i j) -> p g i j", j=15)
                        nc.vector.tensor_tensor(
                            out1,
                            u_g[:, :, 0:14, None].to_broadcast((P, GB, 14, 15)),
                            v_g[:, :, None, :].to_broadcast((P, GB, 14, 15)), ADD)
                        nc.vector.tensor_tensor(
                            rb_g[:, :, 210:SLOTS],
                            u_g[:, :, 14:15].to_broadcast((P, GB, 14)),
                            v_g[:, :, 0:14], ADD)
                        nc.scalar.activation(r_t[:, fs, :], rb[:], AF.Relu)

                    pagg = pa.tile([P, H], F32, tag="agg")
                    z_tiles = []
                    for et in range(NCHUNK):
                        pt = ps.tile([P, H], F32, tag="mm")
                        for kp in range(4):
                            lhs = r_t[:, 2 * kp:2 * kp + 2, et * P:(et + 1) * P]
                            nc.tensor.matmul(pt[:, 0:512], lhs,
                                             w1_s[:, 2 * kp:2 * kp + 2, 0:512],
                                             start=(kp == 0), stop=False, perf_mode=DR)
                            nc.tensor.matmul(pt[:, 512:1024], lhs,
                                             w1_s[:, 2 * kp:2 * kp + 2, 512:1024],
                                             start=(kp == 0), stop=False, perf_mode=DR)
                        # bias b1 as a K=1 rank-1 update
                        nc.tensor.matmul(pt[:, 0:512], ones8[:], b1_r[:, 0:512],
                                         start=False, stop=True)
                        nc.tensor.matmul(pt[:, 512:1024], ones8[:], b1_r[:, 512:1024],
                                         start=False, stop=True)
                        if et % 2 == 0:
                            z_pair = zp.tile([P, 2, H], F8, tag="z")
                            z_tiles.append(z_pair)
                        z_t = z_tiles[et // 2][:, et % 2, :]
                        # LayerNorm (mean is exactly 0 by W1 centering): var only
                        st6 = stp.tile([P, 12], F32, tag="st6")
                        nc.vector.bn_stats(st6[:, 0:6], pt[:, 0:512])
                        nc.vector.bn_stats(st6[:, 6:12], pt[:, 512:1024])
                        mv = stp.tile([P, 2], F32, tag="mv")
                        nc.vector.bn_aggr(mv[:], st6[:].rearrange("p (a b) -> p a b", b=6))
                        sc = stp.tile([P, 1], F32, tag="sc")
                        nc.scalar.activation(sc[:], mv[:, 1:2],
                                             AF.Abs_reciprocal_sqrt, bias=eps_t[:])
                        if trivial_affine_e:
                            nc.scalar.activation(z_t[:], pt[:], AF.Relu, scale=sc[:])
                        else:
                            zn = stp.tile([P, H], F32, tag="zn")
                            nc.scalar.activation(zn[:], pt[:], AF.Identity, scale=sc[:])
                            nc.vector.tensor_tensor(zn[:], zn[:], eg_b[:], mybir.AluOpType.mult)
                            nc.vector.tensor_tensor(zn[:], zn[:], ebe_b[:], ADD)
                            nc.scalar.activation(z_t[:], zn[:], AF.Relu)
                        # interleave aggregation, trailing the LN pipeline
                        if et % 2 == 1 and et >= 3:
                            emit_agg_pair(pagg, (et - 3) // 2, z_tiles[(et - 3) // 2])
                    emit_agg_pair(pagg, NCHUNK // 2 - 1, z_tiles[NCHUNK // 2 - 1])

                    # evict aggregated block (transposed into sT at node-phase start)
                    s_blk = cpool.tile([P, H], BF16, tag=f"sblk{blk}")
                    s_blks.append(s_blk)
                    nc.scalar.activation(s_blk[:], pagg[:], AF.Identity)

            # ================= NODE PHASE =================
            with (
                tc.tile_pool(name="nact", bufs=1) as na,
                tc.tile_pool(name="nst", bufs=3) as nst,
                tc.tile_pool(name="ps2", bufs=2, space="PSUM") as ps2,
                tc.tile_pool(name="pa2", bufs=2, space="PSUM") as pa2,
            ):
                ident = na.tile([P, P], BF16)
                make_identity(nc, ident)
                # ---- transpose aggregated blocks into sT (batched evicts) ----
                for blk in range(NBLK):
                    ptp = pa2.tile([P, 8, P], BF16, tag="tp")
                    for fs in range(8):
                        nc.tensor.transpose(
                            ptp[:, fs, :],
                            s_blks[blk][:, fs * P:(fs + 1) * P], ident[:])
                    nc.scalar.activation(
                        sT[:, :, blk * P:(blk + 1) * P], ptp[:], AF.Identity)

                # ---- node layer 1 -> hT (transposed out, relu+bias in evict) ----
                hT = na.tile([P, 8, N_ROWS], BF16, tag="hT")
                for m in range(8):
                    pt = ps2.tile([P, H], F32, tag="mm")
                    msl = slice(m * P, (m + 1) * P)
                    for half in (0, 512):
                        sl = slice(half, half + 512)
                        chunks = (
                            [(nw0x_s[:, 2 * kp:2 * kp + 2, msl],
                              xT_s[:, 2 * kp:2 * kp + 2, sl], DR) for kp in range(2)]
                            + [(nw0a_s[:, msl], actT_s[:, sl], None)]
                            + [(nw0s_s[:, ks, msl], sT[:, ks, sl], None) for ks in range(8)]
                        )
                        for ci, (lhs, rhs, pm) in enumerate(chunks):
                            nc.tensor.matmul(pt[:, sl], lhs, rhs,
                                             start=(ci == 0), stop=(ci == len(chunks) - 1),
                                             perf_mode=pm)
                    nc.scalar.activation(hT[:, m, :], pt[:], AF.Relu,
                                         bias=nb0_t[:, m:m + 1], scale=1.0 / W_SCALE)

                # ---- node layer 2 (row-major out) + LN + relu -> z2, transpose, layer 3 ----
                z2T = na.tile([P, 8, N_ROWS], BF16, tag="z2T")
                for rt in range(8):
                    pt = ps2.tile([P, H], F32, tag="mm")
                    for ks in range(8):
                        lhs = hT[:, ks, rt * P:(rt + 1) * P]
                        nc.tensor.matmul(pt[:, 0:512], lhs, nw1_s[:, ks, 0:512],
                                         start=(ks == 0), stop=False)
                        nc.tensor.matmul(pt[:, 512:1024], lhs, nw1_s[:, ks, 512:1024],
                                         start=(ks == 0), stop=False)
                    # bias nb1 as a K=1 rank-1 update
                    nc.tensor.matmul(pt[:, 0:512], ones_row[:], nb1_r[:, 0:512],
                                     start=False, stop=True)
                    nc.tensor.matmul(pt[:, 512:1024], ones_row[:], nb1_r[:, 512:1024],
                                     start=False, stop=True)
                    st6 = nst.tile([P, 12], F32, tag="st6")
                    nc.vector.bn_stats(st6[:, 0:6], pt[:, 0:512])
                    nc.vector.bn_stats(st6[:, 6:12], pt[:, 512:1024])
                    mv = nst.tile([P, 2], F32, tag="mv")
                    nc.vector.bn_aggr(mv[:], st6[:].rearrange("p (a b) -> p a b", b=6))
                    sc = nst.tile([P, 1], F32, tag="sc")
                    nc.scalar.activation(sc[:], mv[:, 1:2],
                                         AF.Abs_reciprocal_sqrt, bias=eps_t[:])
                    z2 = nst.tile([P, H], BF16, tag="z2")
                    if trivial_affine_n:
                        nc.scalar.activation(z2[:], pt[:], AF.Relu, scale=sc[:])
                    else:
                        zn = nst.tile([P, H], F32, tag="zn")
                        nc.scalar.activation(zn[:], pt[:], AF.Identity, scale=sc[:])
                        nc.vector.tensor_tensor(zn[:], zn[:], ng_b[:], mybir.AluOpType.mult)
                        nc.vector.tensor_tensor(zn[:], zn[:], nbe_b[:], ADD)
                        nc.scalar.activation(z2[:], zn[:], AF.Relu)
                    ptp = pa2.tile([P, 8, P], BF16, tag="tp2")
                    for fs in range(8):
                        nc.tensor.transpose(ptp[:, fs, :], z2[:, fs * P:(fs + 1) * P], ident[:])
                    nc.scalar.activation(z2T[:, :, rt * P:(rt + 1) * P], ptp[:], AF.Identity)

                # ---- node layer 3 + bias ----
                out_r = out[:].rearrange("(rt p) d -> p rt d", p=P)
                for rt in range(8):
                    pt = ps2.tile([P, H], F32, tag="mm")
                    for ks in range(8):
                        nc.tensor.matmul(pt[:, 0:D], z2T[:, ks, rt * P:(rt + 1) * P],
                                         nw2_s[:, ks, :], start=(ks == 0), stop=False)
                    nc.tensor.matmul(pt[:, 0:D], ones_row[:], nb2_s[:], start=False, stop=True)
                    outb = nst.tile([P, D], BF16, tag="outb")
                    nc.scalar.activation(outb[:], pt[:, 0:D], AF.Identity,
                                         scale=1.0 / W_SCALE)
                    nc.sync.dma_start(out_r[:, rt, :], outb[:])

    return nc


_PROG_CACHE = {}


def _get_program(trivial_e, trivial_n):
    key = (trivial_e, trivial_n)
    if key not in _PROG_CACHE:
        nc = _build_program(trivial_e, trivial_n)
        nc.finalize()
        _PROG_CACHE[key] = nc
    return _PROG_CACHE[key]


def _row_index(core):
    """Padded per-core row order: 8 blocks x (120 edge rows + 8 tail rows)."""
    idx = []
    for blk in range(NBLK):
        idx.append(np.arange(core * EDGE_ROWS + blk * 120,
                             core * EDGE_ROWS + blk * 120 + 120))
        idx.append(NG * 15 + core * 64 + blk * 8 + np.arange(8))
    return np.concatenate(idx)


def kernel(states, action, e_w0, e_b0, e_w1, e_b1, e_g, e_be, e_w2, e_b2,
           n_w0, n_b0, n_w1, n_b1, n_g, n_be, n_w2, n_b2):
    states = _f32(states)
    action = np.asarray(action).astype(np.int64)
    e_w0, e_b0, e_w1, e_b1 = _f32(e_w0), _f32(e_b0), _f32(e_w1), _f32(e_b1)
    e_g, e_be, e_w2, e_b2 = _f32(e_g), _f32(e_be), _f32(e_w2), _f32(e_b2)
    n_w0, n_b0, n_w1, n_b1 = _f32(n_w0), _f32(n_b0), _f32(n_w1), _f32(n_b1)
    n_g, n_be, n_w2, n_b2 = _f32(n_g), _f32(n_be), _f32(n_w2), _f32(n_b2)

    trivial_e = bool(np.all(e_g == 1.0) and np.all(e_be == 0.0))
    trivial_n = bool(np.all(n_g == 1.0) and np.all(n_be == 0.0))
    nc = _get_program(trivial_e, trivial_n)

    flat = states.reshape(-1, D)                        # [8192, 512]
    # one-hot action vectors per flat row
    av = np.zeros((B, A_DIM * K), dtype=np.float32)
    av[np.arange(B), action] = 1.0
    av = av.reshape(-1, A_DIM)                          # [8192, 20]

    # host-folded weights
    wab = e_w0[0:D] + e_w0[D:2 * D]                     # [512, 1024]
    w0c = e_w0[2 * D:3 * D]
    # row-center e_w1/e_b1 over the output dim: pre-LN mean becomes exactly 0
    e_w1c = e_w1 - e_w1.mean(axis=1, keepdims=True)
    e_b1c = e_b1 - e_b1.mean()
    n_w1c = n_w1 - n_w1.mean(axis=1, keepdims=True)
    n_b1c = n_b1 - n_b1.mean()
    nw0x = n_w0[0:D]
    nw0a = n_w0[D:D + A_DIM]
    n_w0s_part = n_w0[D + A_DIM:]
    nw0s = e_w2 @ n_w0s_part                            # [1024, 1024]
    nw0a21 = np.concatenate([nw0a, (e_b2 @ n_w0s_part).reshape(1, H)], axis=0)

    amat = _build_amat()

    def pkn(w, kt):   # [K, N] -> [P, K/128, N]  (pre-transposed weight layout)
        return np.ascontiguousarray(
            w.reshape(kt, P, w.shape[1]).transpose(1, 0, 2))

    common = {
        "wab": _f8(pkn(wab * UV_SCALE, 4)), "w0c": _f8(pkn(w0c * UV_SCALE, 4)),
        "b0": _f32(e_b0),
        "w1": _f8(pkn(e_w1c * W_SCALE, 8)),
        "b1": _f8((e_b1c * W_SCALE).reshape(1, H)),
        "amat": _f8(np.ascontiguousarray(
            amat.reshape(NCHUNK, P, P).transpose(1, 0, 2))),
        "nw0x": _f8(pkn(nw0x * W_SCALE, 4)),
        "nw0a": _bf16(nw0a21 * W_SCALE),
        "nw0s": _bf16(pkn(nw0s * W_SCALE, 8)), "nb0": _f32(n_b0),
        "nw1": _bf16(pkn(n_w1c * W_SCALE, 8)),
        "nb1": _bf16((n_b1c * W_SCALE).reshape(1, H)),
        "nw2": _bf16(pkn(n_w2 * W_SCALE, 8)),
        "nb2": _bf16(n_b2.reshape(1, D) * W_SCALE),
    }
    if not trivial_e:
        common["e_g"] = _f32(e_g)
        common["e_be"] = _f32(e_be)
    if not trivial_n:
        common["n_g"] = _f32(n_g)
        common["n_be"] = _f32(n_be)

    # per-block indicator: 120 edge rows (deg 14), 8 tail rows (deg 0)
    ind = np.tile(np.concatenate(
        [np.full(120, 14.0, np.float32), np.zeros(8, np.float32)]), NBLK)

    in_maps = []
    row_idx = []
    for c in range(N_CORES):
        idx = _row_index(c)
        row_idx.append(idx)
        x_rows = flat[idx]                              # [1024, 512]
        xt = np.ascontiguousarray(x_rows.T)             # [512, 1024]
        at = np.concatenate([av[idx].T, ind.reshape(1, N_ROWS)], axis=0)
        m = dict(common)
        m["xT"] = _f8(xt.reshape(4, P, N_ROWS).transpose(1, 0, 2))
        m["actT"] = _bf16(at)
        in_maps.append(m)

    res = run_bass_kernel_spmd(nc, in_maps, core_ids=list(range(N_CORES)))
    global LAST_RESULT
    LAST_RESULT = res

    out_full = np.empty((B * K, D), dtype=np.float32)
    for c in range(N_CORES):
        out_full[row_idx[c]] = flat[row_idx[c]] + _f32(res.results[c]["out"])
    return out_full.reshape(B, K, D)
